# revision 3
# baseline (speedup 1.0000x reference)
"""Trainium2 Bass kernel v5e for DenseDilatedKnnGraph (B=4, C=192, N=M=3136, K=9).

Full-width find design (no GPSIMD ISA ops - those have a ~25us dispatch
turnaround on this HW). Per batch: L2-normalize x,y over channels;
dist = cdist(xn, yn) + relative_pos; output top-9 smallest per query row.

Device per 128-row tile:
  PE  : psum chunks = xh0.yn0 + pack.ynstk + xl0.yn0  (3 f32r matmuls/chunk)
  ACT : s = sqrt(psum * scale + 2)                    [d^2 = 2 - 2*cos]
  POOL: neg = rpn - s                                 (4x 784-col segments)
  DVE : 4x max8(784) + 4x find_index8 -> 32 candidates/row
        (values + seg-local indices) staged tile-major
HOST: local idx -> global col (+784*seg), top-9 of 32 by (-v, col), dedup.

Sharding: core i = batch i//2, query-row half i%2 (1568 rows, 13 tiles).
Steady state is paced by DVE/Pool at ~7.3us/tile (both at their
throughput floor); PE warmup matmuls during the DMA fill ramp the
p-state so tile 0 isn't 2x slow.
"""

import numpy as np

import concourse.bacc as bacc
import concourse.mybir as mybir
import concourse.tile as tile
from concourse.bass_utils import run_bass_kernel_spmd

B, C, N, M, K = 4, 192, 3136, 3136, 9
NCORES = 8
NL = N // 2                      # 1568 query rows per core
C0 = 128
TILES = [(t * 128, 128) for t in range(12)] + [(1536, 32)]
NT = len(TILES)                  # 13
CHUNKS = [(i * 512, min(512, M - i * 512)) for i in range((M + 511) // 512)]
NSEG = 4
W = M // NSEG                    # 784

F32 = mybir.dt.float32
F32R = mybir.dt.float32r
U16 = mybir.dt.uint16
Alu = mybir.AluOpType
AF = mybir.ActivationFunctionType


def _build_kernel():
    nc = bacc.Bacc("TRN2", target_bir_lowering=False, debug=False,
                   num_devices=NCORES)
    aps = {}
    aps["x0"] = nc.dram_tensor("x0", [128, NL], F32R,
                               kind="ExternalInput").ap()
    aps["x1"] = nc.dram_tensor("x1", [64, NL], F32R,
                               kind="ExternalInput").ap()
    aps["yn0"] = nc.dram_tensor("yn0", [128, M], F32R,
                                kind="ExternalInput").ap()
    aps["yn1"] = nc.dram_tensor("yn1", [64, M], F32R,
                                kind="ExternalInput").ap()
    aps["xscale"] = nc.dram_tensor("xscale", [128, NT], F32,
                                   kind="ExternalInput").ap()
    aps["rpn"] = nc.dram_tensor("rpn", [NL, M], F32,
                                kind="ExternalInput").ap()
    aps["vout"] = nc.dram_tensor("vout", [128, NT * 32], F32,
                                 kind="ExternalOutput").ap()
    aps["iout"] = nc.dram_tensor("iout", [128, NT * 32], U16,
                                 kind="ExternalOutput").ap()
    with tile.TileContext(nc) as tc:
        _emit(tc, aps)
    nc.compile()
    return nc


def _emit(tc, aps):
    nc = tc.nc
    from contextlib import ExitStack
    with ExitStack() as ctx:
        const_p = ctx.enter_context(tc.tile_pool(name="const", bufs=1))
        x_p = ctx.enter_context(tc.tile_pool(name="x", bufs=1))
        y_p = ctx.enter_context(tc.tile_pool(name="y", bufs=1))
        rp_p = ctx.enter_context(tc.tile_pool(name="rp", bufs=3))
        st_p = ctx.enter_context(tc.tile_pool(name="st", bufs=3))
        neg_p = ctx.enter_context(tc.tile_pool(name="neg", bufs=4))
        ps_p = ctx.enter_context(tc.tile_pool(name="ps", bufs=7, space="PSUM"))

        two_col = const_p.tile([128, 1], F32, tag="two")
        nc.vector.memset(two_col[:, :], 2.0)
        scale_col = const_p.tile([128, NT], F32, tag="scale")
        nc.sync.dma_start(scale_col[:, :], aps["xscale"])
        vstage = const_p.tile([128, NT * 32], F32, tag="vstage")
        istage = const_p.tile([128, NT * 32], U16, tag="istage")

        x0 = x_p.tile([128, NL], F32R, tag="x0")
        x1 = x_p.tile([64, NL], F32R, tag="x1")
        yn0 = y_p.tile([128, M], F32R, tag="yn0")
        yn1 = y_p.tile([64, M], F32R, tag="yn1")

        PIECE = NL // 4
        def stat_dma(p):
            ps_ = slice(p * PIECE, (p + 1) * PIECE)
            nc.sync.dma_start(x0[:, ps_], aps["x0"][:, ps_])
            nc.sync.dma_start(x1[:, ps_], aps["x1"][:, ps_])

        def y_dma(cl, ch):
            for lo_c, sz in CHUNKS[cl:ch]:
                cs = slice(lo_c, lo_c + sz)
                nc.sync.dma_start(yn0[:, cs], aps["yn0"][:, cs])
                nc.sync.dma_start(yn1[:, cs], aps["yn1"][:, cs])

        def rp_dma(ti, split=False):
            lo, rows = TILES[ti]
            rpt = rp_p.tile([128, M], F32, tag="rp", name=f"rpt{ti}")
            if split:
                for s in range(NSEG):
                    sg = slice(s * W, (s + 1) * W)
                    nc.sync.dma_start(rpt[0:rows, sg],
                                      aps["rpn"][lo:lo + rows, sg])
            else:
                nc.sync.dma_start(rpt[0:rows, :], aps["rpn"][lo:lo + rows, :])
            return rpt

        # warmup: ramp the PE p-state during the DMA fill with dummy
        # matmuls on dedicated scratch tiles (uninitialized, results
        # discarded, never read - keeps real DMAs dependency-free).
        wstat = const_p.tile([128, 128], F32, tag="wstat")
        wmov = const_p.tile([128, 64], F32, tag="wmov")
        nc.vector.memset(wstat[:, :], 1.0)
        nc.vector.memset(wmov[:, :], 1.0)
        wsr = wstat.bitcast(F32R)
        wmr = wmov.bitcast(F32R)
        wd = ps_p.tile([128, 64], F32, tag="wd", bufs=1)
        for _ in range(24):
            nc.tensor.matmul(wd[:, 0:64], wsr[:, :], wmr[:, :],
                             start=True, stop=True, skip_group_check=True)

        stat_dma(0)
        y_dma(0, 2)
        rp_q = [rp_dma(0, split=True)]
        y_dma(2, 7)
        rp_q.append(rp_dma(1, split=True))
        for p in range(1, 4):
            stat_dma(p)

        for it in range(NT):
            lo, rows = TILES[it]
            xs = slice(lo, lo + rows)
            rpt = rp_q.pop(0)
            if it + 2 < NT:
                rp_q.append(rp_dma(it + 2))
            s_t = st_p.tile([128, M], F32, tag="s")
            neg = neg_p.tile([128, M], F32, tag="neg")
            for lo_c, sz in CHUNKS:
                cs = slice(lo_c, lo_c + sz)
                pd = ps_p.tile([128, 512], F32, tag="pd")
                nc.tensor.matmul(pd[0:rows, 0:sz], x0[:, xs],
                                 yn0[:, cs], start=True, stop=False)
                nc.tensor.matmul(pd[0:rows, 0:sz], x1[:, xs],
                                 yn1[:, cs], start=False, stop=True)
                nc.scalar.activation(s_t[0:rows, cs], pd[0:rows, 0:sz],
                                     AF.Sqrt, bias=two_col[0:rows, 0:1],
                                     scale=scale_col[0:rows, it:it + 1])

            for s in range(NSEG):
                sg = slice(s * W, (s + 1) * W)
                nc.gpsimd.tensor_tensor(neg[0:rows, sg], rpt[0:rows, sg],
                                        s_t[0:rows, sg], op=Alu.subtract)
            o32 = it * 32
            for s in range(NSEG):
                sg = slice(s * W, (s + 1) * W)
                o8 = slice(o32 + 8 * s, o32 + 8 * s + 8)
                nc.vector.max(vstage[0:rows, o8], neg[0:rows, sg])
            for s in range(NSEG):
                sg = slice(s * W, (s + 1) * W)
                o8 = slice(o32 + 8 * s, o32 + 8 * s + 8)
                nc.vector.max_index(istage[0:rows, o8], vstage[0:rows, o8],
                                    neg[0:rows, sg])

        nc.sync.dma_start(aps["vout"], vstage[:, :])
        nc.sync.dma_start(aps["iout"], istage[:, :])


def _rne_mask(u, drop):
    half = np.uint32((1 << (drop - 1)) - 1)
    lsb = (u >> np.uint32(drop)) & np.uint32(1)
    return (u + half + lsb) & np.uint32((~((1 << drop) - 1)) & 0xFFFFFFFF)


_NC = None


def _get_nc():
    global _NC
    if _NC is None:
        _NC = _build_kernel()
    return _NC


def _prep_xy(x, y):
    xsq = x.astype(np.float64)
    nx = np.sqrt((xsq * xsq).sum(axis=1))          # (B, N)
    ysq = y.astype(np.float64)
    ny = np.sqrt((ysq * ysq).sum(axis=1))
    yn = (y / np.maximum(ny, 1e-12)[:, None, :]).astype(np.float32)
    xscale = (-2.0 / np.maximum(nx, 1e-12)).astype(np.float32)
    return yn, xscale


def _run(inputs, trace=False, trace_kwargs=None):
    x = np.ascontiguousarray(np.asarray(inputs["x"], dtype=np.float32)[..., 0])
    y = np.ascontiguousarray(np.asarray(inputs["y"], dtype=np.float32)[..., 0])
    rp = np.asarray(inputs["relative_pos"], dtype=np.float32)
    assert x.shape == (B, C, N) and y.shape == (B, C, M)
    assert rp.shape == (1, N, M)

    yn, xscale = _prep_xy(x, y)
    rpn = -rp[0]
    in_maps = []
    for i in range(NCORES):
        b, h = i // 2, i % 2
        sl = slice(h * NL, (h + 1) * NL)
        xsc_t = np.zeros((128, NT), dtype=np.float32)
        sc = xscale[b, sl]
        for t, (lo, rows) in enumerate(TILES):
            xsc_t[0:rows, t] = sc[lo:lo + rows]
        in_maps.append({
            "x0": np.ascontiguousarray(x[b, 0:C0, sl]),
            "x1": np.ascontiguousarray(x[b, C0:C, sl]),
            "yn0": np.ascontiguousarray(yn[b, 0:C0, :]),
            "yn1": np.ascontiguousarray(yn[b, C0:C, :]),
            "xscale": xsc_t,
            "rpn": np.ascontiguousarray(rpn[sl, :]),
        })
    nc = _get_nc()
    kwargs = {}
    if trace:
        kwargs = dict(trace=True, trace_cores=list(range(NCORES)),
                      trace_kwargs=trace_kwargs or {})
    res = run_bass_kernel_spmd(nc, in_maps, core_ids=list(range(NCORES)),
                               **kwargs)

    nn = np.empty((B, N, K), dtype=np.int32)
    seg_of = (np.arange(32) // 8) * W
    for i in range(NCORES):
        b, h = i // 2, i % 2
        vout = res.results[i]["vout"]
        iout = res.results[i]["iout"]
        V = np.empty((NL, 32), dtype=np.float32)
        COLS = np.empty((NL, 32), dtype=np.int64)
        for t, (lo, rows) in enumerate(TILES):
            o = t * 32
            V[lo:lo + rows] = vout[0:rows, o:o + 32]
            COLS[lo:lo + rows] = iout[0:rows, o:o + 32].astype(
                np.int64) + seg_of[None, :]
        key = (-V).astype(np.float64) + COLS * (2.0 ** -40)
        order = np.argsort(key, axis=1, kind="stable")
        cs = np.take_along_axis(COLS, order, axis=1)
        dup = np.zeros((NL, 32), dtype=bool)
        for j in range(1, 32):
            dup[:, j] = (cs[:, j:j + 1] == cs[:, :j]).any(axis=1)
        rank = np.cumsum(~dup, axis=1)
        out9 = np.empty((NL, K), dtype=np.int32)
        for k in range(1, K + 1):
            jk = np.argmax(rank == k, axis=1)
            out9[:, k - 1] = cs[np.arange(NL), jk]
        nn[b, h * NL:(h + 1) * NL, :] = out9
    center = np.broadcast_to(np.arange(N, dtype=np.int32)[None, :, None],
                             (B, N, K))
    out = np.stack((nn, center), axis=0)
    return out, res


def kernel(**inputs):
    out, _ = _run(inputs, trace=False)
    return out


# revision 4
# speedup vs baseline: 1.1323x; 1.1323x over previous
"""Trainium2 Bass kernel v5e for DenseDilatedKnnGraph (B=4, C=192, N=M=3136, K=9).

Full-width find design (no GPSIMD ISA ops - those have a ~25us dispatch
turnaround on this HW). Per batch: L2-normalize x,y over channels;
dist = cdist(xn, yn) + relative_pos; output top-9 smallest per query row.

Device per 128-row tile:
  PE  : psum chunks = xh0.yn0 + pack.ynstk + xl0.yn0  (3 f32r matmuls/chunk)
  ACT : s = sqrt(psum * scale + 2)                    [d^2 = 2 - 2*cos]
  POOL: neg = rpn - s                                 (4x 784-col segments)
  DVE : 4x max8(784) + 4x find_index8 -> 32 candidates/row
        (values + seg-local indices) staged tile-major
HOST: local idx -> global col (+784*seg), top-9 of 32 by (-v, col), dedup.

Sharding: core i = batch i//2, query-row half i%2 (1568 rows, 13 tiles).
Steady state is paced by DVE/Pool at ~7.3us/tile (both at their
throughput floor); PE warmup matmuls during the DMA fill ramp the
p-state so tile 0 isn't 2x slow.
"""

import numpy as np

import concourse.bacc as bacc
import concourse.mybir as mybir
import concourse.tile as tile
from concourse.bass_utils import run_bass_kernel_spmd

B, C, N, M, K = 4, 192, 3136, 3136, 9
NCORES = 8
NL = N // 2                      # 1568 query rows per core
C0 = 128
TILES = [(t * 128, 128) for t in range(12)] + [(1536, 32)]
NT = len(TILES)                  # 13
CHUNKS = [(i * 512, min(512, M - i * 512)) for i in range((M + 511) // 512)]
NSEG = 4
W = M // NSEG                    # 784

F32 = mybir.dt.float32
F32R = mybir.dt.float32r
U16 = mybir.dt.uint16
Alu = mybir.AluOpType
AF = mybir.ActivationFunctionType


def _build_kernel():
    nc = bacc.Bacc("TRN2", target_bir_lowering=False, debug=False,
                   num_devices=NCORES)
    aps = {}
    aps["xh0"] = nc.dram_tensor("xh0", [128, NL], F32R,
                                kind="ExternalInput").ap()
    aps["pack"] = nc.dram_tensor("pack", [128, NL], F32R,
                                 kind="ExternalInput").ap()
    aps["xl0"] = nc.dram_tensor("xl0", [128, NL], F32R,
                                kind="ExternalInput").ap()
    aps["yn0"] = nc.dram_tensor("yn0", [128, M], F32R,
                                kind="ExternalInput").ap()
    aps["ystk"] = nc.dram_tensor("ystk", [128, M], F32R,
                                 kind="ExternalInput").ap()
    aps["xscale"] = nc.dram_tensor("xscale", [128, NT], F32,
                                   kind="ExternalInput").ap()
    aps["rpn"] = nc.dram_tensor("rpn", [NL, M], F32,
                                kind="ExternalInput").ap()
    aps["vout"] = nc.dram_tensor("vout", [128, NT * 32], F32,
                                 kind="ExternalOutput").ap()
    aps["iout"] = nc.dram_tensor("iout", [128, NT * 32], U16,
                                 kind="ExternalOutput").ap()
    with tile.TileContext(nc) as tc:
        _emit(tc, aps)
    nc.compile()
    return nc


def _emit(tc, aps):
    nc = tc.nc
    from contextlib import ExitStack
    with ExitStack() as ctx:
        const_p = ctx.enter_context(tc.tile_pool(name="const", bufs=1))
        x_p = ctx.enter_context(tc.tile_pool(name="x", bufs=1))
        y_p = ctx.enter_context(tc.tile_pool(name="y", bufs=1))
        rp_p = ctx.enter_context(tc.tile_pool(name="rp", bufs=3))
        st_p = ctx.enter_context(tc.tile_pool(name="st", bufs=3))
        neg_p = ctx.enter_context(tc.tile_pool(name="neg", bufs=4))
        ps_p = ctx.enter_context(tc.tile_pool(name="ps", bufs=7, space="PSUM"))

        two_col = const_p.tile([128, 1], F32, tag="two")
        nc.vector.memset(two_col[:, :], 2.0)
        scale_col = const_p.tile([128, NT], F32, tag="scale")
        nc.sync.dma_start(scale_col[:, :], aps["xscale"])
        vstage = const_p.tile([128, NT * 32], F32, tag="vstage")
        istage = const_p.tile([128, NT * 32], U16, tag="istage")

        xh0 = x_p.tile([128, NL], F32R, tag="xh0")
        pack = x_p.tile([128, NL], F32R, tag="pack")
        xl0 = x_p.tile([128, NL], F32R, tag="xl0")
        yn0 = y_p.tile([128, M], F32R, tag="yn0")
        ynstk = y_p.tile([128, M], F32R, tag="ynstk")

        PIECE = NL // 4
        def stat_dma(p):
            ps_ = slice(p * PIECE, (p + 1) * PIECE)
            nc.sync.dma_start(xh0[:, ps_], aps["xh0"][:, ps_])
            nc.sync.dma_start(pack[:, ps_], aps["pack"][:, ps_])
            nc.sync.dma_start(xl0[:, ps_], aps["xl0"][:, ps_])

        def y_dma(cl, ch):
            for lo_c, sz in CHUNKS[cl:ch]:
                cs = slice(lo_c, lo_c + sz)
                nc.sync.dma_start(yn0[:, cs], aps["yn0"][:, cs])
                nc.sync.dma_start(ynstk[:, cs], aps["ystk"][:, cs])

        def rp_dma(ti, split=False):
            lo, rows = TILES[ti]
            rpt = rp_p.tile([128, M], F32, tag="rp", name=f"rpt{ti}")
            if split:
                for s in range(NSEG):
                    sg = slice(s * W, (s + 1) * W)
                    nc.sync.dma_start(rpt[0:rows, sg],
                                      aps["rpn"][lo:lo + rows, sg])
            else:
                nc.sync.dma_start(rpt[0:rows, :], aps["rpn"][lo:lo + rows, :])
            return rpt

        # warmup: ramp the PE p-state during the DMA fill with dummy
        # matmuls on dedicated scratch tiles (uninitialized, results
        # discarded, never read - keeps real DMAs dependency-free).
        wstat = const_p.tile([128, 128], F32, tag="wstat")
        wmov = const_p.tile([128, 64], F32, tag="wmov")
        nc.vector.memset(wstat[:, :], 1.0)
        nc.vector.memset(wmov[:, :], 1.0)
        wsr = wstat.bitcast(F32R)
        wmr = wmov.bitcast(F32R)
        wd = ps_p.tile([128, 64], F32, tag="wd", bufs=1)
        for _ in range(24):
            nc.tensor.matmul(wd[:, 0:64], wsr[:, :], wmr[:, :],
                             start=True, stop=True, skip_group_check=True)

        stat_dma(0)
        y_dma(0, 2)
        rp_q = [rp_dma(0, split=True)]
        y_dma(2, 7)
        rp_q.append(rp_dma(1, split=True))
        for p in range(1, 4):
            stat_dma(p)

        for it in range(NT):
            lo, rows = TILES[it]
            xs = slice(lo, lo + rows)
            rpt = rp_q.pop(0)
            if it + 2 < NT:
                rp_q.append(rp_dma(it + 2))
            s_t = st_p.tile([128, M], F32, tag="s")
            neg = neg_p.tile([128, M], F32, tag="neg")
            for lo_c, sz in CHUNKS:
                cs = slice(lo_c, lo_c + sz)
                pd = ps_p.tile([128, 512], F32, tag="pd")
                nc.tensor.matmul(pd[0:rows, 0:sz], xh0[:, xs],
                                 yn0[:, cs], start=True, stop=False)
                nc.tensor.matmul(pd[0:rows, 0:sz], pack[:, xs],
                                 ynstk[:, cs], start=False, stop=False)
                nc.tensor.matmul(pd[0:rows, 0:sz], xl0[:, xs],
                                 yn0[:, cs], start=False, stop=True)
                nc.scalar.activation(s_t[0:rows, cs], pd[0:rows, 0:sz],
                                     AF.Sqrt, bias=two_col[0:rows, 0:1],
                                     scale=scale_col[0:rows, it:it + 1])

            for s in range(NSEG):
                sg = slice(s * W, (s + 1) * W)
                nc.gpsimd.tensor_tensor(neg[0:rows, sg], rpt[0:rows, sg],
                                        s_t[0:rows, sg], op=Alu.subtract)
            o32 = it * 32
            for s in range(NSEG):
                sg = slice(s * W, (s + 1) * W)
                o8 = slice(o32 + 8 * s, o32 + 8 * s + 8)
                nc.vector.max(vstage[0:rows, o8], neg[0:rows, sg])
            for s in range(NSEG):
                sg = slice(s * W, (s + 1) * W)
                o8 = slice(o32 + 8 * s, o32 + 8 * s + 8)
                nc.vector.max_index(istage[0:rows, o8], vstage[0:rows, o8],
                                    neg[0:rows, sg])

        nc.sync.dma_start(aps["vout"], vstage[:, :])
        nc.sync.dma_start(aps["iout"], istage[:, :])


def _rne_mask(u, drop):
    half = np.uint32((1 << (drop - 1)) - 1)
    lsb = (u >> np.uint32(drop)) & np.uint32(1)
    return (u + half + lsb) & np.uint32((~((1 << drop) - 1)) & 0xFFFFFFFF)


_NC = None


def _get_nc():
    global _NC
    if _NC is None:
        _NC = _build_kernel()
    return _NC


def _prep_xy(x, y):
    xsq = x.astype(np.float64)
    nx = np.sqrt((xsq * xsq).sum(axis=1))          # (B, N)
    u = x.view(np.uint32)
    xh = _rne_mask(u, 16).view(np.float32)
    xl = x - xh
    xl10 = _rne_mask(xl.view(np.uint32), 13).view(np.float32)
    ysq = y.astype(np.float64)
    ny = np.sqrt((ysq * ysq).sum(axis=1))
    yn = (y / np.maximum(ny, 1e-12)[:, None, :]).astype(np.float32)
    xscale = (-2.0 / np.maximum(nx, 1e-12)).astype(np.float32)
    return xh, xl10, yn, xscale


def _run(inputs, trace=False, trace_kwargs=None):
    x = np.ascontiguousarray(np.asarray(inputs["x"], dtype=np.float32)[..., 0])
    y = np.ascontiguousarray(np.asarray(inputs["y"], dtype=np.float32)[..., 0])
    rp = np.asarray(inputs["relative_pos"], dtype=np.float32)
    assert x.shape == (B, C, N) and y.shape == (B, C, M)
    assert rp.shape == (1, N, M)

    xh, xl10, yn, xscale = _prep_xy(x, y)
    rpn = -rp[0]
    in_maps = []
    for i in range(NCORES):
        b, h = i // 2, i % 2
        sl = slice(h * NL, (h + 1) * NL)
        xsc_t = np.zeros((128, NT), dtype=np.float32)
        sc = xscale[b, sl]
        for t, (lo, rows) in enumerate(TILES):
            xsc_t[0:rows, t] = sc[lo:lo + rows]
        in_maps.append({
            "xh0": np.ascontiguousarray(xh[b, 0:C0, sl]),
            "pack": np.ascontiguousarray(
                np.concatenate([xh[b, C0:C, sl], xl10[b, C0:C, sl]], axis=0)),
            "xl0": np.ascontiguousarray(xl10[b, 0:C0, sl]),
            "yn0": np.ascontiguousarray(yn[b, 0:C0, :]),
            "ystk": np.ascontiguousarray(
                np.concatenate([yn[b, C0:C, :], yn[b, C0:C, :]], axis=0)),
            "xscale": xsc_t,
            "rpn": np.ascontiguousarray(rpn[sl, :]),
        })
    nc = _get_nc()
    kwargs = {}
    if trace:
        kwargs = dict(trace=True, trace_cores=list(range(NCORES)),
                      trace_kwargs=trace_kwargs or {})
    res = run_bass_kernel_spmd(nc, in_maps, core_ids=list(range(NCORES)),
                               **kwargs)

    nn = np.empty((B, N, K), dtype=np.int32)
    seg_of = (np.arange(32) // 8) * W
    for i in range(NCORES):
        b, h = i // 2, i % 2
        vout = res.results[i]["vout"]
        iout = res.results[i]["iout"]
        V = np.empty((NL, 32), dtype=np.float32)
        COLS = np.empty((NL, 32), dtype=np.int64)
        for t, (lo, rows) in enumerate(TILES):
            o = t * 32
            V[lo:lo + rows] = vout[0:rows, o:o + 32]
            COLS[lo:lo + rows] = iout[0:rows, o:o + 32].astype(
                np.int64) + seg_of[None, :]
        key = (-V).astype(np.float64) + COLS * (2.0 ** -40)
        order = np.argsort(key, axis=1, kind="stable")
        cs = np.take_along_axis(COLS, order, axis=1)
        dup = np.zeros((NL, 32), dtype=bool)
        for j in range(1, 32):
            dup[:, j] = (cs[:, j:j + 1] == cs[:, :j]).any(axis=1)
        rank = np.cumsum(~dup, axis=1)
        out9 = np.empty((NL, K), dtype=np.int32)
        for k in range(1, K + 1):
            jk = np.argmax(rank == k, axis=1)
            out9[:, k - 1] = cs[np.arange(NL), jk]
        nn[b, h * NL:(h + 1) * NL, :] = out9
    center = np.broadcast_to(np.arange(N, dtype=np.int32)[None, :, None],
                             (B, N, K))
    out = np.stack((nn, center), axis=0)
    return out, res


def kernel(**inputs):
    out, _ = _run(inputs, trace=False)
    return out


# revision 6
# speedup vs baseline: 1.1675x; 1.0311x over previous
"""Trainium2 Bass kernel v5e for DenseDilatedKnnGraph (B=4, C=192, N=M=3136, K=9).

Full-width find design (no GPSIMD ISA ops - those have a ~25us dispatch
turnaround on this HW). Per batch: L2-normalize x,y over channels;
dist = cdist(xn, yn) + relative_pos; output top-9 smallest per query row.

Device per 128-row tile:
  PE  : psum chunks = x0.yn0 + [x1/2;x1/2].[yn1;yn1]  (2 f32r matmuls/chunk)
  ACT : s = sqrt(psum * scale + 2)                    [d^2 = 2 - 2*cos]
  POOL: neg = rpn - s                                 (4x 784-col segments)
  DVE : 4x max8(784) + 4x find_index8 -> 32 candidates/row
        (values + seg-local indices) staged tile-major
HOST: local idx -> global col (+784*seg), top-9 of 32 by (-v, col), dedup.

Sharding: core i = batch i//2, query-row half i%2 (1568 rows, 13 tiles).
Steady state is paced by DVE/Pool at ~7.3us/tile (both at their
throughput floor); PE warmup matmuls during the DMA fill ramp the
p-state so tile 0 isn't 2x slow.
"""

import numpy as np

import concourse.bacc as bacc
import concourse.mybir as mybir
import concourse.tile as tile
from concourse.bass_utils import run_bass_kernel_spmd

B, C, N, M, K = 4, 192, 3136, 3136, 9
NCORES = 8
NL = N // 2                      # 1568 query rows per core
C0 = 128
TILES = [(t * 128, 128) for t in range(12)] + [(1536, 32)]
NT = len(TILES)                  # 13
CHUNKS = [(i * 512, min(512, M - i * 512)) for i in range((M + 511) // 512)]
NSEG = 4
W = M // NSEG                    # 784

F32 = mybir.dt.float32
F32R = mybir.dt.float32r
U16 = mybir.dt.uint16
Alu = mybir.AluOpType
AF = mybir.ActivationFunctionType


def _build_kernel():
    nc = bacc.Bacc("TRN2", target_bir_lowering=False, debug=False,
                   num_devices=NCORES)
    aps = {}
    aps["xh0"] = nc.dram_tensor("xh0", [128, NL], F32R,
                                kind="ExternalInput").ap()
    aps["pack"] = nc.dram_tensor("pack", [128, NL], F32R,
                                 kind="ExternalInput").ap()
    aps["yn0"] = nc.dram_tensor("yn0", [128, M], F32R,
                                kind="ExternalInput").ap()
    aps["ystk"] = nc.dram_tensor("ystk", [128, M], F32R,
                                 kind="ExternalInput").ap()
    aps["xscale"] = nc.dram_tensor("xscale", [128, NT], F32,
                                   kind="ExternalInput").ap()
    aps["rpn"] = nc.dram_tensor("rpn", [NL, M], F32,
                                kind="ExternalInput").ap()
    aps["vout"] = nc.dram_tensor("vout", [128, NT * 32], F32,
                                 kind="ExternalOutput").ap()
    aps["iout"] = nc.dram_tensor("iout", [128, NT * 32], U16,
                                 kind="ExternalOutput").ap()
    with tile.TileContext(nc) as tc:
        _emit(tc, aps)
    nc.compile()
    return nc


def _emit(tc, aps):
    nc = tc.nc
    from contextlib import ExitStack
    with ExitStack() as ctx:
        const_p = ctx.enter_context(tc.tile_pool(name="const", bufs=1))
        x_p = ctx.enter_context(tc.tile_pool(name="x", bufs=1))
        y_p = ctx.enter_context(tc.tile_pool(name="y", bufs=1))
        rp_p = ctx.enter_context(tc.tile_pool(name="rp", bufs=3))
        st_p = ctx.enter_context(tc.tile_pool(name="st", bufs=3))
        neg_p = ctx.enter_context(tc.tile_pool(name="neg", bufs=4))
        ps_p = ctx.enter_context(tc.tile_pool(name="ps", bufs=7, space="PSUM"))

        two_col = const_p.tile([128, 1], F32, tag="two")
        nc.vector.memset(two_col[:, :], 2.0)
        scale_col = const_p.tile([128, NT], F32, tag="scale")
        nc.sync.dma_start(scale_col[:, :], aps["xscale"])
        vstage = const_p.tile([128, NT * 32], F32, tag="vstage")
        istage = const_p.tile([128, NT * 32], U16, tag="istage")

        xh0 = x_p.tile([128, NL], F32R, tag="xh0")
        pack = x_p.tile([128, NL], F32R, tag="pack")
        yn0 = y_p.tile([128, M], F32R, tag="yn0")
        ynstk = y_p.tile([128, M], F32R, tag="ynstk")

        PIECE = NL // 4
        def stat_dma(p):
            ps_ = slice(p * PIECE, (p + 1) * PIECE)
            nc.sync.dma_start(xh0[:, ps_], aps["xh0"][:, ps_])
            nc.sync.dma_start(pack[:, ps_], aps["pack"][:, ps_])

        def y_dma(cl, ch):
            for lo_c, sz in CHUNKS[cl:ch]:
                cs = slice(lo_c, lo_c + sz)
                nc.sync.dma_start(yn0[:, cs], aps["yn0"][:, cs])
                nc.sync.dma_start(ynstk[:, cs], aps["ystk"][:, cs])

        def rp_dma(ti, split=False):
            lo, rows = TILES[ti]
            rpt = rp_p.tile([128, M], F32, tag="rp", name=f"rpt{ti}")
            if split:
                for s in range(NSEG):
                    sg = slice(s * W, (s + 1) * W)
                    nc.sync.dma_start(rpt[0:rows, sg],
                                      aps["rpn"][lo:lo + rows, sg])
            else:
                nc.sync.dma_start(rpt[0:rows, :], aps["rpn"][lo:lo + rows, :])
            return rpt

        # warmup: ramp the PE p-state during the DMA fill with dummy
        # matmuls on dedicated scratch tiles (uninitialized, results
        # discarded, never read - keeps real DMAs dependency-free).
        wstat = const_p.tile([128, 128], F32, tag="wstat")
        wmov = const_p.tile([128, 64], F32, tag="wmov")
        nc.vector.memset(wstat[:, :], 1.0)
        nc.vector.memset(wmov[:, :], 1.0)
        wsr = wstat.bitcast(F32R)
        wmr = wmov.bitcast(F32R)
        wd = ps_p.tile([128, 64], F32, tag="wd", bufs=1)
        for _ in range(24):
            nc.tensor.matmul(wd[:, 0:64], wsr[:, :], wmr[:, :],
                             start=True, stop=True, skip_group_check=True)

        stat_dma(0)
        y_dma(0, 2)
        rp_q = [rp_dma(0, split=True)]
        y_dma(2, 7)
        rp_q.append(rp_dma(1, split=True))
        for p in range(1, 4):
            stat_dma(p)

        for it in range(NT):
            lo, rows = TILES[it]
            xs = slice(lo, lo + rows)
            rpt = rp_q.pop(0)
            if it + 2 < NT:
                rp_q.append(rp_dma(it + 2))
            s_t = st_p.tile([128, M], F32, tag="s")
            neg = neg_p.tile([128, M], F32, tag="neg")
            for lo_c, sz in CHUNKS:
                cs = slice(lo_c, lo_c + sz)
                pd = ps_p.tile([128, 512], F32, tag="pd")
                nc.tensor.matmul(pd[0:rows, 0:sz], xh0[:, xs],
                                 yn0[:, cs], start=True, stop=False)
                nc.tensor.matmul(pd[0:rows, 0:sz], pack[:, xs],
                                 ynstk[:, cs], start=False, stop=True)
                nc.scalar.activation(s_t[0:rows, cs], pd[0:rows, 0:sz],
                                     AF.Sqrt, bias=two_col[0:rows, 0:1],
                                     scale=scale_col[0:rows, it:it + 1])

            for s in range(NSEG):
                sg = slice(s * W, (s + 1) * W)
                nc.gpsimd.tensor_tensor(neg[0:rows, sg], rpt[0:rows, sg],
                                        s_t[0:rows, sg], op=Alu.subtract)
            o32 = it * 32
            for s in range(NSEG):
                sg = slice(s * W, (s + 1) * W)
                o8 = slice(o32 + 8 * s, o32 + 8 * s + 8)
                nc.vector.max(vstage[0:rows, o8], neg[0:rows, sg])
            for s in range(NSEG):
                sg = slice(s * W, (s + 1) * W)
                o8 = slice(o32 + 8 * s, o32 + 8 * s + 8)
                nc.vector.max_index(istage[0:rows, o8], vstage[0:rows, o8],
                                    neg[0:rows, sg])

        nc.sync.dma_start(aps["vout"], vstage[:, :])
        nc.sync.dma_start(aps["iout"], istage[:, :])


def _rne_mask(u, drop):
    half = np.uint32((1 << (drop - 1)) - 1)
    lsb = (u >> np.uint32(drop)) & np.uint32(1)
    return (u + half + lsb) & np.uint32((~((1 << drop) - 1)) & 0xFFFFFFFF)


_NC = None


def _get_nc():
    global _NC
    if _NC is None:
        _NC = _build_kernel()
    return _NC


def _prep_xy(x, y):
    xsq = x.astype(np.float64)
    nx = np.sqrt((xsq * xsq).sum(axis=1))          # (B, N)
    ysq = y.astype(np.float64)
    ny = np.sqrt((ysq * ysq).sum(axis=1))
    yn = (y / np.maximum(ny, 1e-12)[:, None, :]).astype(np.float32)
    xscale = (-2.0 / np.maximum(nx, 1e-12)).astype(np.float32)
    return yn, xscale


def _run(inputs, trace=False, trace_kwargs=None):
    x = np.ascontiguousarray(np.asarray(inputs["x"], dtype=np.float32)[..., 0])
    y = np.ascontiguousarray(np.asarray(inputs["y"], dtype=np.float32)[..., 0])
    rp = np.asarray(inputs["relative_pos"], dtype=np.float32)
    assert x.shape == (B, C, N) and y.shape == (B, C, M)
    assert rp.shape == (1, N, M)

    yn, xscale = _prep_xy(x, y)
    rpn = -rp[0]
    in_maps = []
    for i in range(NCORES):
        b, h = i // 2, i % 2
        sl = slice(h * NL, (h + 1) * NL)
        xsc_t = np.zeros((128, NT), dtype=np.float32)
        sc = xscale[b, sl]
        for t, (lo, rows) in enumerate(TILES):
            xsc_t[0:rows, t] = sc[lo:lo + rows]
        xhalf = 0.5 * x[b, C0:C, sl]
        in_maps.append({
            "xh0": np.ascontiguousarray(x[b, 0:C0, sl]),
            "pack": np.ascontiguousarray(
                np.concatenate([xhalf, xhalf], axis=0)),
            "yn0": np.ascontiguousarray(yn[b, 0:C0, :]),
            "ystk": np.ascontiguousarray(
                np.concatenate([yn[b, C0:C, :], yn[b, C0:C, :]], axis=0)),
            "xscale": xsc_t,
            "rpn": np.ascontiguousarray(rpn[sl, :]),
        })
    nc = _get_nc()
    kwargs = {}
    if trace:
        kwargs = dict(trace=True, trace_cores=list(range(NCORES)),
                      trace_kwargs=trace_kwargs or {})
    res = run_bass_kernel_spmd(nc, in_maps, core_ids=list(range(NCORES)),
                               **kwargs)

    nn = np.empty((B, N, K), dtype=np.int32)
    seg_of = (np.arange(32) // 8) * W
    for i in range(NCORES):
        b, h = i // 2, i % 2
        vout = res.results[i]["vout"]
        iout = res.results[i]["iout"]
        V = np.empty((NL, 32), dtype=np.float32)
        COLS = np.empty((NL, 32), dtype=np.int64)
        for t, (lo, rows) in enumerate(TILES):
            o = t * 32
            V[lo:lo + rows] = vout[0:rows, o:o + 32]
            COLS[lo:lo + rows] = iout[0:rows, o:o + 32].astype(
                np.int64) + seg_of[None, :]
        key = (-V).astype(np.float64) + COLS * (2.0 ** -40)
        order = np.argsort(key, axis=1, kind="stable")
        cs = np.take_along_axis(COLS, order, axis=1)
        dup = np.zeros((NL, 32), dtype=bool)
        for j in range(1, 32):
            dup[:, j] = (cs[:, j:j + 1] == cs[:, :j]).any(axis=1)
        rank = np.cumsum(~dup, axis=1)
        out9 = np.empty((NL, K), dtype=np.int32)
        for k in range(1, K + 1):
            jk = np.argmax(rank == k, axis=1)
            out9[:, k - 1] = cs[np.arange(NL), jk]
        nn[b, h * NL:(h + 1) * NL, :] = out9
    center = np.broadcast_to(np.arange(N, dtype=np.int32)[None, :, None],
                             (B, N, K))
    out = np.stack((nn, center), axis=0)
    return out, res


def kernel(**inputs):
    out, _ = _run(inputs, trace=False)
    return out


# revision 7
# speedup vs baseline: 1.1927x; 1.0216x over previous
"""Trainium2 Bass kernel v5e for DenseDilatedKnnGraph (B=4, C=192, N=M=3136, K=9).

Full-width find design (no GPSIMD ISA ops - those have a ~25us dispatch
turnaround on this HW). Per batch: L2-normalize x,y over channels;
dist = cdist(xn, yn) + relative_pos; output top-9 smallest per query row.

Device per 128-row tile:
  PE  : psum chunks = x0.yn0 + [x1/2;x1/2].[yn1;yn1]  (2 f32r matmuls/chunk)
  ACT : s = sqrt(psum * scale + 2)                    [d^2 = 2 - 2*cos]
  POOL: neg = rpn - s                                 (4x 784-col segments)
  DVE : 4x max8(784) + 4x find_index8 -> 32 candidates/row
        (values + seg-local indices) staged tile-major
HOST: local idx -> global col (+784*seg), top-9 of 32 by (-v, col), dedup.

Sharding: core i = batch i//2, query-row half i%2 (1568 rows, 13 tiles).
Steady state is paced by DVE/Pool at ~7.3us/tile (both at their
throughput floor); PE warmup matmuls during the DMA fill ramp the
p-state so tile 0 isn't 2x slow.
"""

import numpy as np

import concourse.bacc as bacc
import concourse.mybir as mybir
import concourse.tile as tile
from concourse.bass_utils import run_bass_kernel_spmd

B, C, N, M, K = 4, 192, 3136, 3136, 9
NCORES = 8
NL = N // 2                      # 1568 query rows per core
C0 = 128
TILES = [(t * 128, 128) for t in range(12)] + [(1536, 32)]
NT = len(TILES)                  # 13
CHUNKS = [(i * 512, min(512, M - i * 512)) for i in range((M + 511) // 512)]
NSEG = 4
W = M // NSEG                    # 784

F32 = mybir.dt.float32
F32R = mybir.dt.float32r
U16 = mybir.dt.uint16
Alu = mybir.AluOpType
AF = mybir.ActivationFunctionType


def _build_kernel():
    nc = bacc.Bacc("TRN2", target_bir_lowering=False, debug=False,
                   num_devices=NCORES)
    aps = {}
    aps["xh0"] = nc.dram_tensor("xh0", [128, NL], F32R,
                                kind="ExternalInput").ap()
    aps["pack"] = nc.dram_tensor("pack", [128, NL], F32R,
                                 kind="ExternalInput").ap()
    aps["yn0"] = nc.dram_tensor("yn0", [128, M], F32R,
                                kind="ExternalInput").ap()
    aps["ystk"] = nc.dram_tensor("ystk", [128, M], F32R,
                                 kind="ExternalInput").ap()
    aps["xscale"] = nc.dram_tensor("xscale", [128, NT], F32,
                                   kind="ExternalInput").ap()
    aps["rpn"] = nc.dram_tensor("rpn", [NL, M], F32,
                                kind="ExternalInput").ap()
    aps["vout"] = nc.dram_tensor("vout", [128, NT * 32], F32,
                                 kind="ExternalOutput").ap()
    aps["iout"] = nc.dram_tensor("iout", [128, NT * 32], U16,
                                 kind="ExternalOutput").ap()
    with tile.TileContext(nc) as tc:
        _emit(tc, aps)
    nc.compile()
    return nc


def _emit(tc, aps):
    nc = tc.nc
    from contextlib import ExitStack
    with ExitStack() as ctx:
        const_p = ctx.enter_context(tc.tile_pool(name="const", bufs=1))
        x_p = ctx.enter_context(tc.tile_pool(name="x", bufs=1))
        y_p = ctx.enter_context(tc.tile_pool(name="y", bufs=1))
        rp_p = ctx.enter_context(tc.tile_pool(name="rp", bufs=4))
        st_p = ctx.enter_context(tc.tile_pool(name="st", bufs=3))
        neg_p = ctx.enter_context(tc.tile_pool(name="neg", bufs=4))
        ps_p = ctx.enter_context(tc.tile_pool(name="ps", bufs=7, space="PSUM"))

        two_col = const_p.tile([128, 1], F32, tag="two")
        nc.vector.memset(two_col[:, :], 2.0)
        scale_col = const_p.tile([128, NT], F32, tag="scale")
        nc.sync.dma_start(scale_col[:, :], aps["xscale"])
        vstage = const_p.tile([128, NT * 32], F32, tag="vstage")
        istage = const_p.tile([128, NT * 32], U16, tag="istage")

        xh0 = x_p.tile([128, NL], F32R, tag="xh0")
        pack = x_p.tile([128, NL], F32R, tag="pack")
        yn0 = y_p.tile([128, M], F32R, tag="yn0")
        ynstk = y_p.tile([128, M], F32R, tag="ynstk")

        PIECE = NL // 4
        def stat_dma(p):
            ps_ = slice(p * PIECE, (p + 1) * PIECE)
            nc.sync.dma_start(xh0[:, ps_], aps["xh0"][:, ps_])
            nc.sync.dma_start(pack[:, ps_], aps["pack"][:, ps_])

        def y_dma(cl, ch):
            for lo_c, sz in CHUNKS[cl:ch]:
                cs = slice(lo_c, lo_c + sz)
                nc.sync.dma_start(yn0[:, cs], aps["yn0"][:, cs])
                nc.sync.dma_start(ynstk[:, cs], aps["ystk"][:, cs])

        def rp_dma(ti, split=False):
            lo, rows = TILES[ti]
            rpt = rp_p.tile([128, M], F32, tag="rp", name=f"rpt{ti}")
            if split:
                for s in range(NSEG):
                    sg = slice(s * W, (s + 1) * W)
                    nc.sync.dma_start(rpt[0:rows, sg],
                                      aps["rpn"][lo:lo + rows, sg])
            else:
                nc.sync.dma_start(rpt[0:rows, :], aps["rpn"][lo:lo + rows, :])
            return rpt

        # warmup: ramp the PE p-state during the DMA fill with dummy
        # matmuls on dedicated scratch tiles (uninitialized, results
        # discarded, never read - keeps real DMAs dependency-free).
        wstat = const_p.tile([128, 128], F32, tag="wstat")
        wmov = const_p.tile([128, 64], F32, tag="wmov")
        nc.vector.memset(wstat[:, :], 1.0)
        nc.vector.memset(wmov[:, :], 1.0)
        wsr = wstat.bitcast(F32R)
        wmr = wmov.bitcast(F32R)
        wd = ps_p.tile([128, 64], F32, tag="wd", bufs=1)
        for _ in range(24):
            nc.tensor.matmul(wd[:, 0:64], wsr[:, :], wmr[:, :],
                             start=True, stop=True, skip_group_check=True)

        stat_dma(0)
        y_dma(0, 2)
        rp_q = [rp_dma(0, split=True)]
        y_dma(2, 7)
        rp_q.append(rp_dma(1, split=True))
        rp_q.append(rp_dma(2))
        for p in range(1, 4):
            stat_dma(p)

        for it in range(NT):
            lo, rows = TILES[it]
            xs = slice(lo, lo + rows)
            rpt = rp_q.pop(0)
            if it + 3 < NT:
                rp_q.append(rp_dma(it + 3))
            s_t = st_p.tile([128, M], F32, tag="s")
            neg = neg_p.tile([128, M], F32, tag="neg")
            for lo_c, sz in CHUNKS:
                cs = slice(lo_c, lo_c + sz)
                pd = ps_p.tile([128, 512], F32, tag="pd")
                nc.tensor.matmul(pd[0:rows, 0:sz], xh0[:, xs],
                                 yn0[:, cs], start=True, stop=False)
                nc.tensor.matmul(pd[0:rows, 0:sz], pack[:, xs],
                                 ynstk[:, cs], start=False, stop=True)
                nc.scalar.activation(s_t[0:rows, cs], pd[0:rows, 0:sz],
                                     AF.Sqrt, bias=two_col[0:rows, 0:1],
                                     scale=scale_col[0:rows, it:it + 1])

            for s in range(NSEG):
                sg = slice(s * W, (s + 1) * W)
                nc.gpsimd.tensor_tensor(neg[0:rows, sg], rpt[0:rows, sg],
                                        s_t[0:rows, sg], op=Alu.subtract)
            o32 = it * 32
            for s in range(NSEG):
                sg = slice(s * W, (s + 1) * W)
                o8 = slice(o32 + 8 * s, o32 + 8 * s + 8)
                nc.vector.max(vstage[0:rows, o8], neg[0:rows, sg])
            for s in range(NSEG):
                sg = slice(s * W, (s + 1) * W)
                o8 = slice(o32 + 8 * s, o32 + 8 * s + 8)
                nc.vector.max_index(istage[0:rows, o8], vstage[0:rows, o8],
                                    neg[0:rows, sg])

        nc.sync.dma_start(aps["vout"][:, 0:11 * 32], vstage[:, 0:11 * 32])
        nc.sync.dma_start(aps["iout"][:, 0:11 * 32], istage[:, 0:11 * 32])
        nc.sync.dma_start(aps["vout"][:, 11 * 32:], vstage[:, 11 * 32:])
        nc.sync.dma_start(aps["iout"][:, 11 * 32:], istage[:, 11 * 32:])


def _rne_mask(u, drop):
    half = np.uint32((1 << (drop - 1)) - 1)
    lsb = (u >> np.uint32(drop)) & np.uint32(1)
    return (u + half + lsb) & np.uint32((~((1 << drop) - 1)) & 0xFFFFFFFF)


_NC = None


def _get_nc():
    global _NC
    if _NC is None:
        _NC = _build_kernel()
    return _NC


def _prep_xy(x, y):
    xsq = x.astype(np.float64)
    nx = np.sqrt((xsq * xsq).sum(axis=1))          # (B, N)
    ysq = y.astype(np.float64)
    ny = np.sqrt((ysq * ysq).sum(axis=1))
    yn = (y / np.maximum(ny, 1e-12)[:, None, :]).astype(np.float32)
    xscale = (-2.0 / np.maximum(nx, 1e-12)).astype(np.float32)
    return yn, xscale


def _run(inputs, trace=False, trace_kwargs=None):
    x = np.ascontiguousarray(np.asarray(inputs["x"], dtype=np.float32)[..., 0])
    y = np.ascontiguousarray(np.asarray(inputs["y"], dtype=np.float32)[..., 0])
    rp = np.asarray(inputs["relative_pos"], dtype=np.float32)
    assert x.shape == (B, C, N) and y.shape == (B, C, M)
    assert rp.shape == (1, N, M)

    yn, xscale = _prep_xy(x, y)
    rpn = -rp[0]
    in_maps = []
    for i in range(NCORES):
        b, h = i // 2, i % 2
        sl = slice(h * NL, (h + 1) * NL)
        xsc_t = np.zeros((128, NT), dtype=np.float32)
        sc = xscale[b, sl]
        for t, (lo, rows) in enumerate(TILES):
            xsc_t[0:rows, t] = sc[lo:lo + rows]
        xhalf = 0.5 * x[b, C0:C, sl]
        in_maps.append({
            "xh0": np.ascontiguousarray(x[b, 0:C0, sl]),
            "pack": np.ascontiguousarray(
                np.concatenate([xhalf, xhalf], axis=0)),
            "yn0": np.ascontiguousarray(yn[b, 0:C0, :]),
            "ystk": np.ascontiguousarray(
                np.concatenate([yn[b, C0:C, :], yn[b, C0:C, :]], axis=0)),
            "xscale": xsc_t,
            "rpn": np.ascontiguousarray(rpn[sl, :]),
        })
    nc = _get_nc()
    kwargs = {}
    if trace:
        kwargs = dict(trace=True, trace_cores=list(range(NCORES)),
                      trace_kwargs=trace_kwargs or {})
    res = run_bass_kernel_spmd(nc, in_maps, core_ids=list(range(NCORES)),
                               **kwargs)

    nn = np.empty((B, N, K), dtype=np.int32)
    seg_of = (np.arange(32) // 8) * W
    for i in range(NCORES):
        b, h = i // 2, i % 2
        vout = res.results[i]["vout"]
        iout = res.results[i]["iout"]
        V = np.empty((NL, 32), dtype=np.float32)
        COLS = np.empty((NL, 32), dtype=np.int64)
        for t, (lo, rows) in enumerate(TILES):
            o = t * 32
            V[lo:lo + rows] = vout[0:rows, o:o + 32]
            COLS[lo:lo + rows] = iout[0:rows, o:o + 32].astype(
                np.int64) + seg_of[None, :]
        key = (-V).astype(np.float64) + COLS * (2.0 ** -40)
        order = np.argsort(key, axis=1, kind="stable")
        cs = np.take_along_axis(COLS, order, axis=1)
        dup = np.zeros((NL, 32), dtype=bool)
        for j in range(1, 32):
            dup[:, j] = (cs[:, j:j + 1] == cs[:, :j]).any(axis=1)
        rank = np.cumsum(~dup, axis=1)
        out9 = np.empty((NL, K), dtype=np.int32)
        for k in range(1, K + 1):
            jk = np.argmax(rank == k, axis=1)
            out9[:, k - 1] = cs[np.arange(NL), jk]
        nn[b, h * NL:(h + 1) * NL, :] = out9
    center = np.broadcast_to(np.arange(N, dtype=np.int32)[None, :, None],
                             (B, N, K))
    out = np.stack((nn, center), axis=0)
    return out, res


def kernel(**inputs):
    out, _ = _run(inputs, trace=False)
    return out
